# revision 2
# baseline (speedup 1.0000x reference)
"""Trainium2 Bass kernel for nn_MessagePassing (gnn_message_passing) — v3.

Math (per batch b = core b):
    coef[s,e] = sum_o adj[s,o] * edge[s,o,e]
    v[s,e,i]  = sum_j W[e,i,j] * node[s,j]
    out[s,i]  = sum_e coef[s,e] * v[s,e,i]

v3 = v2.6 + tail restructure: tile 7's FIRST o-half is loaded at the front
of the DMA ring and processed early (~30us); only its second 2MB half plus
a short DVE chain remain after the wire drains (~11us tail vs ~21us).
"""

import numpy as np
from contextlib import ExitStack

import concourse.bass as bass
import concourse.bacc as bacc
import concourse.mybir as mybir
import concourse.tile as tile
from concourse.bass_utils import run_bass_kernel_spmd
from concourse.masks import make_identity

B, N, D, E = 8, 1024, 128, 8
P = 128
NT = N // P  # 8 s-tiles per core
H = N // 2

F32 = mybir.dt.float32
BF16 = mybir.dt.bfloat16
I32 = mybir.dt.int32
MUL = mybir.AluOpType.mult
ADD = mybir.AluOpType.add
LAST = NT - 1


def build_nc():
    nc = bacc.Bacc("TRN2", target_bir_lowering=False, debug=False, num_devices=B)

    node_d = nc.dram_tensor("node_state", [N, D], F32, kind="ExternalInput").ap()
    edge_d = nc.dram_tensor("edge_type_mat", [N, N, E], F32, kind="ExternalInput").ap()
    adj_d = nc.dram_tensor("adj_mat", [N, N], F32, kind="ExternalInput").ap()
    w_d = nc.dram_tensor("W", [E, D, D], F32, kind="ExternalInput").ap()
    out_d = nc.dram_tensor("out", [N, D], F32, kind="ExternalOutput").ap()

    with tile.TileContext(nc) as tc, ExitStack() as ctx:
        const_pool = ctx.enter_context(tc.tile_pool(name="const", bufs=1))
        edge_pool = ctx.enter_context(tc.tile_pool(name="edge", bufs=8))
        quad_pool = ctx.enter_context(tc.tile_pool(name="quad", bufs=2))
        work_pool = ctx.enter_context(tc.tile_pool(name="work", bufs=2))
        psv_pool = ctx.enter_context(tc.tile_pool(name="psv", bufs=4, space="PSUM"))
        pss_pool = ctx.enter_context(tc.tile_pool(name="pss", bufs=2, space="PSUM"))

        # ---- all loads upfront on the SWDGE ring; ring order:
        #   e0a e0b a0 | e7a a7 | e1 a1 node w | e2 a2 ... e6 a6 | e7b ----
        edge_tiles = {}
        adj_r = adj_d.rearrange("(t p) o -> p t o", p=P)
        adj_tiles = [
            const_pool.tile([P, N], BF16, name=f"adj{t}") for t in range(NT)
        ]
        node_all = const_pool.tile([P, NT, D], F32)
        w_all = const_pool.tile([P, E, D], F32)  # [i, e, j]

        def load_edge(t, half=False):
            # each edge tile is followed on the ring by its adj slice
            et = edge_pool.tile([P, N, E], BF16, tag="edge_t")
            if not half:
                nc.gpsimd.dma_start(et[:], edge_d[bass.ts(t, P)])
            else:
                nc.gpsimd.dma_start(et[:, 0:H, :], edge_d[t * P : t * P + P, 0:H])
                nc.gpsimd.dma_start(et[:, H:N, :], edge_d[t * P : t * P + P, H:N])
            nc.gpsimd.dma_start(adj_tiles[t][:], adj_r[:, t, :])
            return et

        # identity FIRST: it is built on the GpSimd queue, which the upfront
        # dma_start instructions otherwise occupy for ~60us (ring-space pacing)
        ident = const_pool.tile([P, P], F32)
        make_identity(nc, ident[:])
        ident_bf = const_pool.tile([P, P], BF16)
        nc.vector.tensor_copy(ident_bf[:], ident[:])

        edge_tiles[0] = load_edge(0, half=True)
        # tile 7 first half + its adj ride at the FRONT of the ring
        et7 = edge_pool.tile([P, N, E], BF16, tag="edge_t", name="et7")
        nc.gpsimd.dma_start(et7[:, 0:H, :], edge_d[LAST * P : LAST * P + P, 0:H])
        nc.gpsimd.dma_start(adj_tiles[LAST][:], adj_r[:, LAST, :])
        edge_tiles[LAST] = et7
        edge_tiles[1] = load_edge(1)
        nc.gpsimd.dma_start(node_all[:], node_d.rearrange("(t p) j -> p t j", p=P))
        nc.gpsimd.dma_start(w_all[:], w_d.rearrange("e i j -> i e j"))
        for t in range(2, NT - 1):
            edge_tiles[t] = load_edge(t)
        # tile 7 second half is the very last transfer
        nc.gpsimd.dma_start(et7[:, H:N, :], edge_d[LAST * P : LAST * P + P, H:N])

        # node^T[j, s] and W[e]^T[j, i] via PE transpose
        nodeT = const_pool.tile([P, N], F32)
        for t in range(NT):
            pt = pss_pool.tile([P, P], F32, tag="ps_small")
            nc.tensor.transpose(pt[:], node_all[:, t, :], ident[:])
            nc.scalar.copy(nodeT[:, bass.ts(t, P)], pt[:])
        wT = const_pool.tile([P, E, D], F32)  # [j, e, i]
        for e in range(E):
            pt = pss_pool.tile([P, P], F32, tag="ps_small")
            nc.tensor.transpose(pt[:], w_all[:, e, :], ident[:])
            nc.scalar.copy(wT[:, e, :], pt[:])

        coef7a = const_pool.tile([P, E], F32)
        coef7b = const_pool.tile([P, E], F32)

        state = {}

        def deint_and_stt(edge_t, t, k, halves, quad, coef_dst):
            """ScalarE quad de-interleave + 8 DVE coef STTs for o-range k."""
            h = N // halves
            nc.scalar.copy(
                quad[:, :, k * h : (k + 1) * h, :].bitcast(I32),
                edge_t[:, k * h : (k + 1) * h, :]
                .bitcast(I32)
                .rearrange("p n (q t) -> p q n t", q=2),
            )
            scratch = work_pool.tile([P, N], BF16, tag="scratch")
            for e in range(E):
                q, j = divmod(e, 4)
                nc.vector.scalar_tensor_tensor(
                    out=scratch[:, 0:h],
                    in0=quad[:, q, k * h : (k + 1) * h, j],
                    scalar=1.0,
                    in1=adj_tiles[t][:, k * h : (k + 1) * h],
                    op0=MUL,
                    op1=MUL,
                    accum_out=coef_dst[:, e : e + 1],
                )

        def v_matmuls(t):
            psums = []
            for g in range(E // 4):
                pv = psv_pool.tile([P, 4, D], F32, tag="psum_v")
                nc.tensor.matmul(
                    pv[:],
                    lhsT=nodeT[:, bass.ts(t, P)],
                    rhs=wT[:, g * 4 : (g + 1) * 4, :],
                    start=True,
                    stop=True,
                )
                psums.append(pv)
            return psums

        def stage_compute(t):
            edge_t = edge_tiles.pop(t)
            halves = 2 if t == 0 else 1
            coef = work_pool.tile([P, E], F32, tag="coef")
            coef_b = work_pool.tile([P, E], F32, tag="coef_b")
            quad = quad_pool.tile([P, 2, N, 4], BF16, tag="quad")
            for k in range(halves):
                deint_and_stt(edge_t, t, k, halves, quad, coef if k == 0 else coef_b)
            if halves == 2:
                nc.vector.tensor_add(coef[:], coef[:], coef_b[:])
            state[t] = (coef, v_matmuls(t))

        def stage_reduce(t):
            """Sc: sv_e = v_e * coef_e; PE: psum-accumulate the 8 sv_e."""
            coef, psums = state.pop(t)
            sv = work_pool.tile([P, E, D], BF16, tag="sv")
            for e in range(E):
                nc.scalar.mul(sv[:, e, :], psums[e // 4][:, e % 4, :], coef[:, e : e + 1])
            acc = pss_pool.tile([P, D], F32, tag="ps_small")
            for e in range(E):
                nc.tensor.matmul(
                    acc[:],
                    lhsT=ident_bf[:],
                    rhs=sv[:, e, :],
                    start=(e == 0),
                    stop=(e == E - 1),
                )
            out_sb = work_pool.tile([P, D], F32, tag="out_sb")
            nc.scalar.copy(out_sb[:], acc[:])
            nc.scalar.dma_start(out_d[bass.ts(t, P)], out_sb[:])

        # ---- software pipeline ----
        stage_compute(0)
        # tile 7 first half: runs right after tile 0 while the wire streams
        quad7 = quad_pool.tile([P, 2, N, 4], BF16, tag="quad")
        deint_and_stt(et7, LAST, 0, 2, quad7, coef7a)

        for t in range(1, NT - 1):
            stage_compute(t)
            stage_reduce(t - 1)

        # tile 7 second half: the only work left after the wire drains
        quad7b = quad_pool.tile([P, 2, N, 4], BF16, tag="quad")
        deint_and_stt(et7, LAST, 1, 2, quad7b, coef7b)
        nc.vector.tensor_add(coef7b[:], coef7b[:], coef7a[:])
        psums7 = v_matmuls(LAST)
        stage_reduce(NT - 2)

        # shortest-tail reduce for tile 7: direct DVE chain
        acc_a = work_pool.tile([P, D], F32, tag="acc_a")
        acc_b = work_pool.tile([P, D], F32, tag="acc_b")
        nc.vector.tensor_scalar_mul(acc_a[:], psums7[0][:, 0, :], coef7b[:, 0:1])
        cur, nxt = acc_a, acc_b
        for e in range(1, E):
            nc.vector.scalar_tensor_tensor(
                out=nxt[:],
                in0=psums7[e // 4][:, e % 4, :],
                scalar=coef7b[:, e : e + 1],
                in1=cur[:],
                op0=MUL,
                op1=ADD,
            )
            cur, nxt = nxt, cur
        nc.scalar.dma_start(out_d[bass.ts(LAST, P)], cur[:])

    nc.compile()
    return nc


_NC_CACHE = None


def get_nc():
    global _NC_CACHE
    if _NC_CACHE is None:
        _NC_CACHE = build_nc()
    return _NC_CACHE


def make_in_maps(node_state, edge_type_mat, adj_mat, W):
    return [
        {
            "node_state": np.ascontiguousarray(node_state[b], dtype=np.float32),
            "edge_type_mat": np.ascontiguousarray(edge_type_mat[b], dtype=np.float32),
            "adj_mat": np.ascontiguousarray(adj_mat[b], dtype=np.float32),
            "W": np.ascontiguousarray(W, dtype=np.float32),
        }
        for b in range(B)
    ]


def kernel(node_state, edge_type_mat, adj_mat, W):
    nc = get_nc()
    in_maps = make_in_maps(node_state, edge_type_mat, adj_mat, W)
    res = run_bass_kernel_spmd(nc, in_maps, list(range(B)))
    return np.stack([res.results[b]["out"] for b in range(B)], axis=0)
